# revision 13
# baseline (speedup 1.0000x reference)
"""AliasFreeActivation Trainium2 kernel (v3: fp16 matmuls, banded down-path).

out = crop10(down2(leaky_relu(up4(x + bias)) * sqrt2))   [4,256,236,236]

Decomposition per (batch,channel) image (1024 images, 128 per core):
  leaky_relu(t)*s = 0.6*s*t + 0.4*s*|t|   (slope 0.2)
so with y = up4(xb):
  out = Down(0.4*sqrt2*|y|)  +  Down(0.6*sqrt2*y)
The second (linear) term collapses through the composed matrices
Mv = A@D so it never touches the big upsampled grid.

Stages (matmul contraction is always the SBUF partition dim; the image
data is the stationary lhsT so the kept axis lands on the output
partitions, chaining without transposes):
  s1  v1[w,ho]   = sum_h xb[h,w] A[h,ho]            1 MM  N=512
  sA  u1[w,hd]   = sum_h xb[h,w] Mv[h,hd]           1 MM  N=256   (linear)
  s2  p2[ho,wo]  = sum_w v1[w,ho] A2[w,wo]          4 MM  N=512   (A2=0.4*sqrt2*A)
  abs Y = |p2|                                      (one ACT/DVE pass)
  s3  z[wo,hd]   = sum_ho Y[ho,wo] D[ho,hd]        16 MM  banded N<=70
  s4  o[hd,wd]   = sum_wo z[wo,hd] D[wo,wd]         8 MM  banded N<=70
  sB  o += sum_w u1[w,hd] Mh[w,wd]                  2 MM  N=236   (Mh=0.6*sqrt2*Mv)
All matmul operands are fp16 (1 cycle/row at any N, FWL weight loads);
PSUM accumulation is fp32.
"""
import numpy as np

UP, DOWN, MARGIN, NEG_SLOPE = 4, 2, 10, 0.2
SQRT2 = 1.4142135623730951
H = W = 128
OUT = 236
NCORES = 8
NIMG = 128

# down-matrix window per 128-row K-chunk: D[s,o] nonzero for s in [2o-5,2o+6]
DWIN = [(0, 67), (61, 131), (125, 195), (189, 256)]
# y's wo extent actually needed by the cropped output: [15, 497) -> 482 cols
WO0, WO1 = 15, 497
WOC = WO1 - WO0                     # 482
WCH = [(0, 128), (128, 256), (256, 384), (384, 482)]   # wo chunks (cropped)
# stage-4 windows: D rows [15+128j ...) -> wd' col windows
DWIN4 = [(5, 74), (69, 138), (133, 202), (197, 251)]

_cache = {}


def _build_nc(nimg=NIMG):
    import concourse.bacc as bacc
    import concourse.bass as bass
    import concourse.tile as tile
    from concourse import mybir

    F32 = mybir.dt.float32
    F16 = mybir.dt.float16
    AF = mybir.ActivationFunctionType
    ALU = mybir.AluOpType

    nc = bacc.Bacc("TRN2", target_bir_lowering=False)
    x_d = nc.dram_tensor("x", [nimg, H, W], F32, kind="ExternalInput")
    b_d = nc.dram_tensor("bias", [nimg], F32, kind="ExternalInput")
    c_d = nc.dram_tensor("cm", [128, 2048], F16, kind="ExternalInput")
    o_d = nc.dram_tensor("out", [nimg, OUT, OUT], F32, kind="ExternalOutput")

    with tile.TileContext(nc) as tc:
        with (
            tc.tile_pool(name="const", bufs=1) as const,
            tc.tile_pool(name="xin", bufs=4) as xin,
            tc.tile_pool(name="xbp", bufs=2) as xbp,
            tc.tile_pool(name="v1p", bufs=2) as v1p,
            tc.tile_pool(name="u1p", bufs=2) as u1p,
            tc.tile_pool(name="yp", bufs=2) as yp,
            tc.tile_pool(name="zp", bufs=2) as zp,
            tc.tile_pool(name="op", bufs=4) as op_,
            tc.tile_pool(name="ps", bufs=2, space="PSUM") as ps,
        ):
            cm = const.tile([128, 2048], F16)
            nc.sync.dma_start(out=cm, in_=c_d[:])
            A_sb = cm[:, 0:512]
            A2_sb = cm[:, 512:512 + WOC]               # 0.4*sqrt2*A, wo-cropped
            dw0 = 994

            def D_sb(k):
                o0, o1 = DWIN[k]
                return cm[:, dw0 + 70 * k: dw0 + 70 * k + (o1 - o0)]

            dw40 = dw0 + 280

            def D4_sb(k):
                o0, o1 = DWIN4[k]
                rows = WCH[k][1] - WCH[k][0]
                return cm[:rows, dw40 + 70 * k: dw40 + 70 * k + (o1 - o0)]

            mv0 = dw40 + 280                           # Mv [128,236] (cropped)
            Mv_sb = cm[:, mv0: mv0 + 236]
            mh0 = mv0 + 236                            # Mh [128,236] (cropped)
            Mh_sb = cm[:, mh0: mh0 + 236]

            bb = const.tile([128, nimg], F32)
            nc.gpsimd.dma_start(
                out=bb,
                in_=bass.AP(tensor=b_d[:].tensor, offset=0,
                            ap=[[0, 128], [1, nimg]]),
            )

            # warm PE's clock on the const DMA lane
            pwarm = ps.tile([128, 256], F32, name="p3")
            nc.tensor.matmul(out=pwarm[:32, :256], lhsT=cm[:, :32],
                             rhs=cm[:, :256], start=True, stop=True)

            for i in range(nimg):
                X = xin.tile([128, W], F32)
                nc.sync.dma_start(out=X, in_=x_d[i])
                Xb = xbp.tile([128, W], F16)
                nc.scalar.activation(out=Xb, in_=X, func=AF.Identity,
                                     bias=bb[:, i:i + 1], scale=1.0)

                # s1: up vertical
                P1 = ps.tile([128, 512], F32, name="p1")
                nc.tensor.matmul(out=P1, lhsT=Xb, rhs=A_sb,
                                 start=True, stop=True)
                V1 = v1p.tile([128, 512], F16)
                nc.vector.tensor_copy(out=V1, in_=P1)

                # sA: linear path, vertical compose
                PA = ps.tile([128, 256], F32, name="p4")
                nc.tensor.matmul(out=PA[:, :OUT], lhsT=Xb, rhs=Mv_sb,
                                 start=True, stop=True)
                U1 = u1p.tile([128, OUT], F16)
                nc.vector.tensor_copy(out=U1, in_=PA[:, :OUT])

                # s2 + |.|: up horizontal then one-pass abs evacuation
                Y = yp.tile([128, 4, WOC], F16)
                for m in range(4):
                    P2 = ps.tile([128, 512], F32, name="p2")
                    nc.tensor.matmul(out=P2[:, :WOC],
                                     lhsT=V1[:, 128 * m:128 * (m + 1)],
                                     rhs=A2_sb, start=True, stop=True)
                    nc.scalar.activation(out=Y[:, m, :], in_=P2[:, :WOC],
                                         func=AF.Abs, bias=0.0, scale=1.0)

                # s3: down vertical (banded); wo chunks 128/128/128/98
                Z = zp.tile([128, 4, OUT], F16)
                for m in range(4):
                    c0, c1 = WCH[m]
                    cnt = c1 - c0
                    P3 = ps.tile([128, 256], F32, name="p3")
                    for k in range(4):
                        o0, o1 = DWIN[k]
                        nc.tensor.matmul(
                            out=P3[:cnt, o0:o1],
                            lhsT=Y[:, k, c0:c1],
                            rhs=D_sb(k), start=(k == 0), stop=(k == 3))
                    nc.vector.tensor_copy(out=Z[:cnt, m, :],
                                          in_=P3[:cnt, MARGIN:MARGIN + OUT])

                # s4 + sB: down horizontal (banded) + linear-path accumulate
                for mo, (h0, h1) in enumerate(((0, 128), (128, OUT))):
                    rows = h1 - h0
                    P4 = ps.tile([128, 256], F32, name="p4")
                    for k in range(4):
                        o0, o1 = DWIN4[k]
                        ck0, ck1 = WCH[k]
                        nc.tensor.matmul(
                            out=P4[:rows, o0:o1],
                            lhsT=Z[:ck1 - ck0, k, h0:h1],
                            rhs=D4_sb(k), start=(k == 0), stop=False)
                    # linear path accumulates into the same PSUM group
                    nc.tensor.matmul(
                        out=P4[:rows, MARGIN:MARGIN + OUT],
                        lhsT=U1[:, h0:h1],
                        rhs=Mh_sb, start=False, stop=True)
                    O = op_.tile([128, OUT], F32)
                    nc.vector.tensor_copy(out=O[:rows, :],
                                          in_=P4[:rows, MARGIN:MARGIN + OUT])
                    nc.sync.dma_start(out=o_d[i, h0:h1, :], in_=O[:rows, :])

    nc.finalize()
    return nc


def _filter_matrices(up_filter, down_filter):
    fu = np.asarray(up_filter, dtype=np.float64)
    fd = np.asarray(down_filter, dtype=np.float64)
    i = np.arange(128)[:, None]
    o = np.arange(512)[None, :]
    t = 10 + o - 4 * i
    A = np.where((t >= 0) & (t < 24), fu[np.clip(t, 0, 23)], 0.0)
    s = np.arange(512)[:, None]
    o2 = np.arange(256)[None, :]
    t2 = 6 + 2 * o2 - s
    D = np.where((t2 >= 0) & (t2 < 12), fd[np.clip(t2, 0, 11)], 0.0)
    return A, D


def _pack_consts(up_filter, down_filter):
    A, D = _filter_matrices(up_filter, down_filter)
    cm = np.zeros((128, 2048), dtype=np.float16)
    cm[:, 0:512] = A.astype(np.float16)
    cm[:, 512:512 + WOC] = (A * (0.4 * SQRT2))[:, WO0:WO1].astype(np.float16)
    dw0 = 994
    for k, (o0, o1) in enumerate(DWIN):
        cm[:, dw0 + 70 * k: dw0 + 70 * k + (o1 - o0)] = \
            D[128 * k:128 * (k + 1), o0:o1].astype(np.float16)
    dw40 = dw0 + 280
    for k, (o0, o1) in enumerate(DWIN4):
        r0 = WO0 + 128 * k
        r1 = min(WO1, r0 + 128)
        cm[:r1 - r0, dw40 + 70 * k: dw40 + 70 * k + (o1 - o0)] = \
            D[r0:r1, o0:o1].astype(np.float16)
    Mv = A @ D
    mv0 = dw40 + 280
    cm[:, mv0: mv0 + 236] = Mv[:, 10:246].astype(np.float16)
    mh0 = mv0 + 236
    cm[:, mh0: mh0 + 236] = (Mv * (0.6 * SQRT2))[:, 10:246].astype(np.float16)
    return cm


def _run(x, bias, up_filter, down_filter, trace=False):
    from concourse.bass_utils import run_bass_kernel_spmd

    if "nc" not in _cache:
        _cache["nc"] = _build_nc()
    nc = _cache["nc"]

    cm = _pack_consts(up_filter, down_filter)
    xf = np.ascontiguousarray(np.asarray(x, dtype=np.float32)
                              .reshape(NCORES * NIMG, H, W))
    bias = np.asarray(bias, dtype=np.float32)
    bias_full = np.tile(bias, (NCORES * NIMG) // bias.shape[0])

    in_maps = []
    for c in range(NCORES):
        in_maps.append({
            "x": xf[NIMG * c: NIMG * (c + 1)],
            "bias": np.ascontiguousarray(bias_full[NIMG * c: NIMG * (c + 1)]),
            "cm": cm,
        })
    res = run_bass_kernel_spmd(nc, in_maps, core_ids=list(range(NCORES)),
                               trace=trace)
    out = np.concatenate([res.results[c]["out"][None] for c in range(NCORES)], 0)
    out = out.reshape(4, 256, OUT, OUT)
    return out, res


def kernel(x, bias, up_filter, down_filter):
    out, _ = _run(x, bias, up_filter, down_filter, trace=False)
    return out


def kernel_traced(x, bias, up_filter, down_filter):
    return _run(x, bias, up_filter, down_filter, trace=True)


# revision 14
# speedup vs baseline: 1.0188x; 1.0188x over previous
"""AliasFreeActivation Trainium2 kernel (v3: fp16 matmuls, banded down-path).

out = crop10(down2(leaky_relu(up4(x + bias)) * sqrt2))   [4,256,236,236]

Decomposition per (batch,channel) image (1024 images, 128 per core):
  leaky_relu(t)*s = 0.6*s*t + 0.4*s*|t|   (slope 0.2)
so with y = up4(xb):
  out = Down(0.4*sqrt2*|y|)  +  Down(0.6*sqrt2*y)
The second (linear) term collapses through the composed matrices
Mv = A@D so it never touches the big upsampled grid.

Stages (matmul contraction is always the SBUF partition dim; the image
data is the stationary lhsT so the kept axis lands on the output
partitions, chaining without transposes):
  s1  v1[w,ho]   = sum_h xb[h,w] A[h,ho]            1 MM  N=512
  sA  u1[w,hd]   = sum_h xb[h,w] Mv[h,hd]           1 MM  N=256   (linear)
  s2  p2[ho,wo]  = sum_w v1[w,ho] A2[w,wo]          4 MM  N=512   (A2=0.4*sqrt2*A)
  abs Y = |p2|                                      (one ACT/DVE pass)
  s3  z[wo,hd]   = sum_ho Y[ho,wo] D[ho,hd]        16 MM  banded N<=70
  s4  o[hd,wd]   = sum_wo z[wo,hd] D[wo,wd]         8 MM  banded N<=70
  sB  o += sum_w u1[w,hd] Mh[w,wd]                  2 MM  N=236   (Mh=0.6*sqrt2*Mv)
All matmul operands are fp16 (1 cycle/row at any N, FWL weight loads);
PSUM accumulation is fp32.
"""
import numpy as np

UP, DOWN, MARGIN, NEG_SLOPE = 4, 2, 10, 0.2
SQRT2 = 1.4142135623730951
H = W = 128
OUT = 236
NCORES = 8
NIMG = 128

# down-matrix window per 128-row K-chunk: D[s,o] nonzero for s in [2o-5,2o+6]
DWIN = [(0, 67), (61, 131), (125, 195), (189, 256)]
# y's wo extent actually needed by the cropped output: [15, 497) -> 482 cols
WO0, WO1 = 15, 497
WOC = WO1 - WO0                     # 482
WCH = [(0, 128), (128, 256), (256, 384), (384, 482)]   # wo chunks (cropped)
# stage-4 windows: D rows [15+128j ...) -> wd' col windows
DWIN4 = [(5, 74), (69, 138), (133, 202), (197, 251)]

_cache = {}


def _build_nc(nimg=NIMG):
    import concourse.bacc as bacc
    import concourse.bass as bass
    import concourse.tile as tile
    from concourse import mybir

    F32 = mybir.dt.float32
    F16 = mybir.dt.float16
    AF = mybir.ActivationFunctionType
    ALU = mybir.AluOpType

    nc = bacc.Bacc("TRN2", target_bir_lowering=False)
    x_d = nc.dram_tensor("x", [nimg, H, W], F32, kind="ExternalInput")
    b_d = nc.dram_tensor("bias", [nimg], F32, kind="ExternalInput")
    c_d = nc.dram_tensor("cm", [128, 2048], F16, kind="ExternalInput")
    o_d = nc.dram_tensor("out", [nimg, OUT, OUT], F32, kind="ExternalOutput")

    with tile.TileContext(nc) as tc:
        with (
            tc.tile_pool(name="const", bufs=1) as const,
            tc.tile_pool(name="xin", bufs=4) as xin,
            tc.tile_pool(name="xbp", bufs=2) as xbp,
            tc.tile_pool(name="v1p", bufs=2) as v1p,
            tc.tile_pool(name="u1p", bufs=2) as u1p,
            tc.tile_pool(name="yp", bufs=2) as yp,
            tc.tile_pool(name="zp", bufs=2) as zp,
            tc.tile_pool(name="op", bufs=4) as op_,
            tc.tile_pool(name="ps", bufs=2, space="PSUM") as ps,
        ):
            cm = const.tile([128, 2048], F16)
            nc.sync.dma_start(out=cm, in_=c_d[:])
            A_sb = cm[:, 0:512]
            A2_sb = cm[:, 512:512 + WOC]               # 0.4*sqrt2*A, wo-cropped
            dw0 = 994

            def D_sb(k):
                o0, o1 = DWIN[k]
                return cm[:, dw0 + 70 * k: dw0 + 70 * k + (o1 - o0)]

            dw40 = dw0 + 280

            def D4_sb(k):
                o0, o1 = DWIN4[k]
                rows = WCH[k][1] - WCH[k][0]
                return cm[:rows, dw40 + 70 * k: dw40 + 70 * k + (o1 - o0)]

            mv0 = dw40 + 280                           # Mv [128,236] (cropped)
            Mv_sb = cm[:, mv0: mv0 + 236]
            mh0 = mv0 + 236                            # Mh [128,236] (cropped)
            Mh_sb = cm[:, mh0: mh0 + 236]

            bb = const.tile([128, nimg], F32)
            nc.gpsimd.dma_start(
                out=bb,
                in_=bass.AP(tensor=b_d[:].tensor, offset=0,
                            ap=[[0, 128], [1, nimg]]),
            )

            # warm PE's clock on the const DMA lane
            pwarm = ps.tile([128, 256], F32, name="p3")
            nc.tensor.matmul(out=pwarm[:32, :256], lhsT=cm[:, :32],
                             rhs=cm[:, :256], start=True, stop=True)

            for i in range(nimg):
                X = xin.tile([128, W], F32)
                nc.sync.dma_start(out=X, in_=x_d[i])
                Xb = xbp.tile([128, W], F16)
                nc.scalar.activation(out=Xb, in_=X, func=AF.Identity,
                                     bias=bb[:, i:i + 1], scale=1.0)

                # s1: up vertical
                P1 = ps.tile([128, 512], F32, name="p1")
                nc.tensor.matmul(out=P1, lhsT=Xb, rhs=A_sb,
                                 start=True, stop=True)
                V1 = v1p.tile([128, 512], F16)
                nc.vector.tensor_copy(out=V1, in_=P1)

                # s2 + |.|: up horizontal then one-pass abs evacuation
                Y = yp.tile([128, 4, WOC], F16)
                for m in range(4):
                    P2 = ps.tile([128, 512], F32, name="p2")
                    nc.tensor.matmul(out=P2[:, :WOC],
                                     lhsT=V1[:, 128 * m:128 * (m + 1)],
                                     rhs=A2_sb, start=True, stop=True)
                    nc.scalar.activation(out=Y[:, m, :], in_=P2[:, :WOC],
                                         func=AF.Abs, bias=0.0, scale=1.0)

                # s3: down vertical (banded); wo chunks 128/128/128/98
                Z = zp.tile([128, 4, OUT], F16)
                for m in range(4):
                    c0, c1 = WCH[m]
                    cnt = c1 - c0
                    P3 = ps.tile([128, 256], F32, name="p3")
                    for k in range(4):
                        o0, o1 = DWIN[k]
                        nc.tensor.matmul(
                            out=P3[:cnt, o0:o1],
                            lhsT=Y[:, k, c0:c1],
                            rhs=D_sb(k), start=(k == 0), stop=(k == 3))
                    nc.vector.tensor_copy(out=Z[:cnt, m, :],
                                          in_=P3[:cnt, MARGIN:MARGIN + OUT])

                # sA: linear path, vertical compose (emitted late so its
                # PSUM slot reuse never stalls the front of the pipeline)
                PA = ps.tile([128, 256], F32, name="p4")
                nc.tensor.matmul(out=PA[:, :OUT], lhsT=Xb, rhs=Mv_sb,
                                 start=True, stop=True)
                U1 = u1p.tile([128, OUT], F16)
                nc.vector.tensor_copy(out=U1, in_=PA[:, :OUT])

                # s4 + sB: down horizontal (banded) + linear-path accumulate
                for mo, (h0, h1) in enumerate(((0, 128), (128, OUT))):
                    rows = h1 - h0
                    P4 = ps.tile([128, 256], F32, name="p4")
                    for k in range(4):
                        o0, o1 = DWIN4[k]
                        ck0, ck1 = WCH[k]
                        nc.tensor.matmul(
                            out=P4[:rows, o0:o1],
                            lhsT=Z[:ck1 - ck0, k, h0:h1],
                            rhs=D4_sb(k), start=(k == 0), stop=False)
                    # linear path accumulates into the same PSUM group
                    nc.tensor.matmul(
                        out=P4[:rows, MARGIN:MARGIN + OUT],
                        lhsT=U1[:, h0:h1],
                        rhs=Mh_sb, start=False, stop=True)
                    O = op_.tile([128, OUT], F32)
                    nc.vector.tensor_copy(out=O[:rows, :],
                                          in_=P4[:rows, MARGIN:MARGIN + OUT])
                    nc.sync.dma_start(out=o_d[i, h0:h1, :], in_=O[:rows, :])

    nc.finalize()
    return nc


def _filter_matrices(up_filter, down_filter):
    fu = np.asarray(up_filter, dtype=np.float64)
    fd = np.asarray(down_filter, dtype=np.float64)
    i = np.arange(128)[:, None]
    o = np.arange(512)[None, :]
    t = 10 + o - 4 * i
    A = np.where((t >= 0) & (t < 24), fu[np.clip(t, 0, 23)], 0.0)
    s = np.arange(512)[:, None]
    o2 = np.arange(256)[None, :]
    t2 = 6 + 2 * o2 - s
    D = np.where((t2 >= 0) & (t2 < 12), fd[np.clip(t2, 0, 11)], 0.0)
    return A, D


def _pack_consts(up_filter, down_filter):
    A, D = _filter_matrices(up_filter, down_filter)
    cm = np.zeros((128, 2048), dtype=np.float16)
    cm[:, 0:512] = A.astype(np.float16)
    cm[:, 512:512 + WOC] = (A * (0.4 * SQRT2))[:, WO0:WO1].astype(np.float16)
    dw0 = 994
    for k, (o0, o1) in enumerate(DWIN):
        cm[:, dw0 + 70 * k: dw0 + 70 * k + (o1 - o0)] = \
            D[128 * k:128 * (k + 1), o0:o1].astype(np.float16)
    dw40 = dw0 + 280
    for k, (o0, o1) in enumerate(DWIN4):
        r0 = WO0 + 128 * k
        r1 = min(WO1, r0 + 128)
        cm[:r1 - r0, dw40 + 70 * k: dw40 + 70 * k + (o1 - o0)] = \
            D[r0:r1, o0:o1].astype(np.float16)
    Mv = A @ D
    mv0 = dw40 + 280
    cm[:, mv0: mv0 + 236] = Mv[:, 10:246].astype(np.float16)
    mh0 = mv0 + 236
    cm[:, mh0: mh0 + 236] = (Mv * (0.6 * SQRT2))[:, 10:246].astype(np.float16)
    return cm


def _run(x, bias, up_filter, down_filter, trace=False):
    from concourse.bass_utils import run_bass_kernel_spmd

    if "nc" not in _cache:
        _cache["nc"] = _build_nc()
    nc = _cache["nc"]

    cm = _pack_consts(up_filter, down_filter)
    xf = np.ascontiguousarray(np.asarray(x, dtype=np.float32)
                              .reshape(NCORES * NIMG, H, W))
    bias = np.asarray(bias, dtype=np.float32)
    bias_full = np.tile(bias, (NCORES * NIMG) // bias.shape[0])

    in_maps = []
    for c in range(NCORES):
        in_maps.append({
            "x": xf[NIMG * c: NIMG * (c + 1)],
            "bias": np.ascontiguousarray(bias_full[NIMG * c: NIMG * (c + 1)]),
            "cm": cm,
        })
    res = run_bass_kernel_spmd(nc, in_maps, core_ids=list(range(NCORES)),
                               trace=trace)
    out = np.concatenate([res.results[c]["out"][None] for c in range(NCORES)], 0)
    out = out.reshape(4, 256, OUT, OUT)
    return out, res


def kernel(x, bias, up_filter, down_filter):
    out, _ = _run(x, bias, up_filter, down_filter, trace=False)
    return out


def kernel_traced(x, bias, up_filter, down_filter):
    return _run(x, bias, up_filter, down_filter, trace=True)


# revision 15
# speedup vs baseline: 1.1847x; 1.1628x over previous
"""AliasFreeActivation Trainium2 kernel (v3: fp16 matmuls, banded down-path).

out = crop10(down2(leaky_relu(up4(x + bias)) * sqrt2))   [4,256,236,236]

Decomposition per (batch,channel) image (1024 images, 128 per core):
  leaky_relu(t)*s = 0.6*s*t + 0.4*s*|t|   (slope 0.2)
so with y = up4(xb):
  out = Down(0.4*sqrt2*|y|)  +  Down(0.6*sqrt2*y)
The second (linear) term collapses through the composed matrices
Mv = A@D so it never touches the big upsampled grid.

Stages (matmul contraction is always the SBUF partition dim; the image
data is the stationary lhsT so the kept axis lands on the output
partitions, chaining without transposes):
  s1  v1[w,ho]   = sum_h xb[h,w] A[h,ho]            1 MM  N=512
  sA  u1[w,hd]   = sum_h xb[h,w] Mv[h,hd]           1 MM  N=256   (linear)
  s2  p2[ho,wo]  = sum_w v1[w,ho] A2[w,wo]          4 MM  N=512   (A2=0.4*sqrt2*A)
  abs Y = |p2|                                      (one ACT/DVE pass)
  s3  z[wo,hd]   = sum_ho Y[ho,wo] D[ho,hd]        16 MM  banded N<=70
  s4  o[hd,wd]   = sum_wo z[wo,hd] D[wo,wd]         8 MM  banded N<=70
  sB  o += sum_w u1[w,hd] Mh[w,wd]                  2 MM  N=236   (Mh=0.6*sqrt2*Mv)
All matmul operands are fp16 (1 cycle/row at any N, FWL weight loads);
PSUM accumulation is fp32.
"""
import numpy as np

UP, DOWN, MARGIN, NEG_SLOPE = 4, 2, 10, 0.2
SQRT2 = 1.4142135623730951
H = W = 128
OUT = 236
NCORES = 8
NIMG = 128

# down-matrix window per 128-row K-chunk: D[s,o] nonzero for s in [2o-5,2o+6]
DWIN = [(0, 67), (61, 131), (125, 195), (189, 256)]

_cache = {}


def _build_nc(nimg=NIMG):
    import concourse.bacc as bacc
    import concourse.bass as bass
    import concourse.tile as tile
    from concourse import mybir

    F32 = mybir.dt.float32
    F16 = mybir.dt.float16
    AF = mybir.ActivationFunctionType
    ALU = mybir.AluOpType

    nc = bacc.Bacc("TRN2", target_bir_lowering=False)
    x_d = nc.dram_tensor("x", [nimg, H, W], F32, kind="ExternalInput")
    b_d = nc.dram_tensor("bias", [nimg], F32, kind="ExternalInput")
    c_d = nc.dram_tensor("cm", [128, 2048], F16, kind="ExternalInput")
    o_d = nc.dram_tensor("out", [nimg, OUT, OUT], F32, kind="ExternalOutput")

    with tile.TileContext(nc) as tc:
        with (
            tc.tile_pool(name="const", bufs=1) as const,
            tc.tile_pool(name="xin", bufs=4) as xin,
            tc.tile_pool(name="xbp", bufs=2) as xbp,
            tc.tile_pool(name="v1p", bufs=2) as v1p,
            tc.tile_pool(name="u1p", bufs=2) as u1p,
            tc.tile_pool(name="yp", bufs=2) as yp,
            tc.tile_pool(name="zp", bufs=2) as zp,
            tc.tile_pool(name="op", bufs=4) as op_,
            tc.tile_pool(name="ps", bufs=2, space="PSUM") as ps,
        ):
            cm = const.tile([128, 2048], F16)
            nc.sync.dma_start(out=cm, in_=c_d[:])
            A_sb = cm[:, 0:512]
            A2_sb = cm[:, 512:1024]                    # 0.4*sqrt2*A
            dw0 = 1024

            def D_sb(k):
                o0, o1 = DWIN[k]
                return cm[:, dw0 + 70 * k: dw0 + 70 * k + (o1 - o0)]

            mv0 = dw0 + 280                            # Mv [128,256]
            Mv_sb = cm[:, mv0: mv0 + 256]
            mh0 = mv0 + 256                            # Mh [128,236] (cropped)
            Mh_sb = cm[:, mh0: mh0 + 236]

            bb = const.tile([128, nimg], F32)
            nc.gpsimd.dma_start(
                out=bb,
                in_=bass.AP(tensor=b_d[:].tensor, offset=0,
                            ap=[[0, 128], [1, nimg]]),
            )

            # warm PE's clock on the const DMA lane
            pwarm = ps.tile([128, 256], F32, name="p3")
            nc.tensor.matmul(out=pwarm[:32, :256], lhsT=cm[:, :32],
                             rhs=cm[:, :256], start=True, stop=True)

            for i in range(nimg):
                X = xin.tile([128, W], F32)
                nc.sync.dma_start(out=X, in_=x_d[i])
                Xb = xbp.tile([128, W], F16)
                nc.scalar.activation(out=Xb, in_=X, func=AF.Identity,
                                     bias=bb[:, i:i + 1], scale=1.0)

                # s1: up vertical
                P1 = ps.tile([128, 512], F32, name="p1")
                nc.tensor.matmul(out=P1, lhsT=Xb, rhs=A_sb,
                                 start=True, stop=True)
                V1 = v1p.tile([128, 512], F16)
                nc.vector.tensor_copy(out=V1, in_=P1)

                # sA: linear path, vertical compose
                PA = ps.tile([128, 256], F32, name="p3")
                nc.tensor.matmul(out=PA, lhsT=Xb, rhs=Mv_sb,
                                 start=True, stop=True)
                U1 = u1p.tile([128, OUT], F16)
                nc.vector.tensor_copy(out=U1, in_=PA[:, MARGIN:MARGIN + OUT])

                # s2 + |.|: up horizontal then one-pass abs evacuation
                Y = yp.tile([128, 4, 512], F16)
                for m in range(4):
                    P2 = ps.tile([128, 512], F32, name="p2")
                    nc.tensor.matmul(out=P2, lhsT=V1[:, 128 * m:128 * (m + 1)],
                                     rhs=A2_sb, start=True, stop=True)
                    nc.scalar.activation(out=Y[:, m, :], in_=P2,
                                         func=AF.Abs, bias=0.0, scale=1.0)

                # s3: down vertical (banded)
                Z = zp.tile([128, 4, OUT], F16)
                for m in range(4):
                    P3 = ps.tile([128, 256], F32, name="p3")
                    for k in range(4):
                        o0, o1 = DWIN[k]
                        nc.tensor.matmul(
                            out=P3[:, o0:o1],
                            lhsT=Y[:, k, 128 * m:128 * (m + 1)],
                            rhs=D_sb(k), start=(k == 0), stop=(k == 3))
                    nc.vector.tensor_copy(out=Z[:, m, :],
                                          in_=P3[:, MARGIN:MARGIN + OUT])

                # s4 + sB: down horizontal (banded) + linear-path accumulate
                for mo, (h0, h1) in enumerate(((0, 128), (128, OUT))):
                    rows = h1 - h0
                    P4 = ps.tile([128, 256], F32, name="p4")
                    for k in range(4):
                        o0, o1 = DWIN[k]
                        nc.tensor.matmul(
                            out=P4[:rows, o0:o1],
                            lhsT=Z[:, k, h0:h1],
                            rhs=D_sb(k), start=(k == 0), stop=False)
                    # linear path accumulates into the same PSUM group
                    nc.tensor.matmul(
                        out=P4[:rows, MARGIN:MARGIN + OUT],
                        lhsT=U1[:, h0:h1],
                        rhs=Mh_sb, start=False, stop=True)
                    O = op_.tile([128, OUT], F32)
                    nc.vector.tensor_copy(out=O[:rows, :],
                                          in_=P4[:rows, MARGIN:MARGIN + OUT])
                    nc.sync.dma_start(out=o_d[i, h0:h1, :], in_=O[:rows, :])

    nc.finalize()
    return nc


def _filter_matrices(up_filter, down_filter):
    fu = np.asarray(up_filter, dtype=np.float64)
    fd = np.asarray(down_filter, dtype=np.float64)
    i = np.arange(128)[:, None]
    o = np.arange(512)[None, :]
    t = 10 + o - 4 * i
    A = np.where((t >= 0) & (t < 24), fu[np.clip(t, 0, 23)], 0.0)
    s = np.arange(512)[:, None]
    o2 = np.arange(256)[None, :]
    t2 = 6 + 2 * o2 - s
    D = np.where((t2 >= 0) & (t2 < 12), fd[np.clip(t2, 0, 11)], 0.0)
    return A, D


def _pack_consts(up_filter, down_filter):
    A, D = _filter_matrices(up_filter, down_filter)
    cm = np.zeros((128, 2048), dtype=np.float16)
    cm[:, 0:512] = A.astype(np.float16)
    cm[:, 512:1024] = (A * (0.4 * SQRT2)).astype(np.float16)
    dw0 = 1024
    for k, (o0, o1) in enumerate(DWIN):
        cm[:, dw0 + 70 * k: dw0 + 70 * k + (o1 - o0)] = \
            D[128 * k:128 * (k + 1), o0:o1].astype(np.float16)
    Mv = A @ D
    mv0 = dw0 + 280
    cm[:, mv0: mv0 + 256] = Mv.astype(np.float16)
    mh0 = mv0 + 256
    cm[:, mh0: mh0 + 236] = (Mv * (0.6 * SQRT2))[:, 10:246].astype(np.float16)
    return cm


def _run(x, bias, up_filter, down_filter, trace=False):
    from concourse.bass_utils import run_bass_kernel_spmd

    if "nc" not in _cache:
        _cache["nc"] = _build_nc()
    nc = _cache["nc"]

    cm = _pack_consts(up_filter, down_filter)
    xf = np.ascontiguousarray(np.asarray(x, dtype=np.float32)
                              .reshape(NCORES * NIMG, H, W))
    bias = np.asarray(bias, dtype=np.float32)
    bias_full = np.tile(bias, (NCORES * NIMG) // bias.shape[0])

    in_maps = []
    for c in range(NCORES):
        in_maps.append({
            "x": xf[NIMG * c: NIMG * (c + 1)],
            "bias": np.ascontiguousarray(bias_full[NIMG * c: NIMG * (c + 1)]),
            "cm": cm,
        })
    res = run_bass_kernel_spmd(nc, in_maps, core_ids=list(range(NCORES)),
                               trace=trace)
    out = np.concatenate([res.results[c]["out"][None] for c in range(NCORES)], 0)
    out = out.reshape(4, 256, OUT, OUT)
    return out, res


def kernel(x, bias, up_filter, down_filter):
    out, _ = _run(x, bias, up_filter, down_filter, trace=False)
    return out


def kernel_traced(x, bias, up_filter, down_filter):
    return _run(x, bias, up_filter, down_filter, trace=True)


# revision 16
# speedup vs baseline: 1.2621x; 1.0653x over previous
"""AliasFreeActivation Trainium2 kernel (v3: fp16 matmuls, banded down-path).

out = crop10(down2(leaky_relu(up4(x + bias)) * sqrt2))   [4,256,236,236]

Decomposition per (batch,channel) image (1024 images, 128 per core):
  leaky_relu(t)*s = 0.6*s*t + 0.4*s*|t|   (slope 0.2)
so with y = up4(xb):
  out = Down(0.4*sqrt2*|y|)  +  Down(0.6*sqrt2*y)
The second (linear) term collapses through the composed matrices
Mv = A@D so it never touches the big upsampled grid.

Stages (matmul contraction is always the SBUF partition dim; the image
data is the stationary lhsT so the kept axis lands on the output
partitions, chaining without transposes):
  s1  v1[w,ho]   = sum_h xb[h,w] A[h,ho]            1 MM  N=512
  sA  u1[w,hd]   = sum_h xb[h,w] Mv[h,hd]           1 MM  N=256   (linear)
  s2  p2[ho,wo]  = sum_w v1[w,ho] A2[w,wo]          4 MM  N=512   (A2=0.4*sqrt2*A)
  abs Y = |p2|                                      (one ACT/DVE pass)
  s3  z[wo,hd]   = sum_ho Y[ho,wo] D[ho,hd]        16 MM  banded N<=70
  s4  o[hd,wd]   = sum_wo z[wo,hd] D[wo,wd]         8 MM  banded N<=70
  sB  o += sum_w u1[w,hd] Mh[w,wd]                  2 MM  N=236   (Mh=0.6*sqrt2*Mv)
All matmul operands are fp16 (1 cycle/row at any N, FWL weight loads);
PSUM accumulation is fp32.
"""
import numpy as np

UP, DOWN, MARGIN, NEG_SLOPE = 4, 2, 10, 0.2
SQRT2 = 1.4142135623730951
H = W = 128
OUT = 236
NCORES = 8
NIMG = 128

# down-matrix window per 128-row K-chunk: D[s,o] nonzero for s in [2o-5,2o+6]
DWIN = [(0, 67), (61, 131), (125, 195), (189, 256)]

_cache = {}


def _build_nc(nimg=NIMG):
    import concourse.bacc as bacc
    import concourse.bass as bass
    import concourse.tile as tile
    from concourse import mybir

    F32 = mybir.dt.float32
    F16 = mybir.dt.float16
    AF = mybir.ActivationFunctionType
    ALU = mybir.AluOpType

    nc = bacc.Bacc("TRN2", target_bir_lowering=False)
    x_d = nc.dram_tensor("x", [nimg, H, W], F32, kind="ExternalInput")
    b_d = nc.dram_tensor("bias", [nimg], F32, kind="ExternalInput")
    c_d = nc.dram_tensor("cm", [128, 2048], F16, kind="ExternalInput")
    o_d = nc.dram_tensor("out", [nimg, OUT, OUT], F32, kind="ExternalOutput")

    with tile.TileContext(nc) as tc:
        with (
            tc.tile_pool(name="const", bufs=1) as const,
            tc.tile_pool(name="xin", bufs=4) as xin,
            tc.tile_pool(name="xbp", bufs=2) as xbp,
            tc.tile_pool(name="v1p", bufs=2) as v1p,
            tc.tile_pool(name="u1p", bufs=2) as u1p,
            tc.tile_pool(name="yp", bufs=2) as yp,
            tc.tile_pool(name="zp", bufs=2) as zp,
            tc.tile_pool(name="op", bufs=4) as op_,
            tc.tile_pool(name="ps", bufs=2, space="PSUM") as ps,
            tc.tile_pool(name="ps3", bufs=3, space="PSUM") as ps3,
            tc.tile_pool(name="ps4", bufs=1, space="PSUM") as ps4,
        ):
            cm = const.tile([128, 2048], F16)
            nc.sync.dma_start(out=cm, in_=c_d[:])
            A_sb = cm[:, 0:512]
            A2_sb = cm[:, 512:1024]                    # 0.4*sqrt2*A
            dw0 = 1024

            def D_sb(k):
                o0, o1 = DWIN[k]
                return cm[:, dw0 + 70 * k: dw0 + 70 * k + (o1 - o0)]

            mv0 = dw0 + 280                            # Mv [128,256]
            Mv_sb = cm[:, mv0: mv0 + 256]
            mh0 = mv0 + 256                            # Mh [128,236] (cropped)
            Mh_sb = cm[:, mh0: mh0 + 236]

            bb = const.tile([128, nimg], F32)
            nc.gpsimd.dma_start(
                out=bb,
                in_=bass.AP(tensor=b_d[:].tensor, offset=0,
                            ap=[[0, 128], [1, nimg]]),
            )

            # warm PE's clock on the const DMA lane
            pwarm = ps3.tile([128, 256], F32, name="p3")
            nc.tensor.matmul(out=pwarm[:32, :256], lhsT=cm[:, :32],
                             rhs=cm[:, :256], start=True, stop=True)

            for i in range(nimg):
                X = xin.tile([128, W], F32)
                nc.sync.dma_start(out=X, in_=x_d[i])
                Xb = xbp.tile([128, W], F16)
                nc.scalar.activation(out=Xb, in_=X, func=AF.Identity,
                                     bias=bb[:, i:i + 1], scale=1.0)

                # s1: up vertical
                P1 = ps.tile([128, 512], F32, name="p1")
                nc.tensor.matmul(out=P1, lhsT=Xb, rhs=A_sb,
                                 start=True, stop=True)
                V1 = v1p.tile([128, 512], F16)
                nc.vector.tensor_copy(out=V1, in_=P1)

                # sA: linear path, vertical compose
                PA = ps3.tile([128, 256], F32, name="p3")
                nc.tensor.matmul(out=PA, lhsT=Xb, rhs=Mv_sb,
                                 start=True, stop=True)
                U1 = u1p.tile([128, OUT], F16)
                nc.vector.tensor_copy(out=U1, in_=PA[:, MARGIN:MARGIN + OUT])

                # s2 + |.|: up horizontal then one-pass abs evacuation
                Y = yp.tile([128, 4, 512], F16)
                for m in range(4):
                    P2 = ps.tile([128, 512], F32, name="p2")
                    nc.tensor.matmul(out=P2, lhsT=V1[:, 128 * m:128 * (m + 1)],
                                     rhs=A2_sb, start=True, stop=True)
                    nc.scalar.activation(out=Y[:, m, :], in_=P2,
                                         func=AF.Abs, bias=0.0, scale=1.0)

                # s3: down vertical (banded)
                Z = zp.tile([128, 4, OUT], F16)
                for m in range(4):
                    P3 = ps3.tile([128, 256], F32, name="p3")
                    for k in range(4):
                        o0, o1 = DWIN[k]
                        nc.tensor.matmul(
                            out=P3[:, o0:o1],
                            lhsT=Y[:, k, 128 * m:128 * (m + 1)],
                            rhs=D_sb(k), start=(k == 0), stop=(k == 3))
                    nc.vector.tensor_copy(out=Z[:, m, :],
                                          in_=P3[:, MARGIN:MARGIN + OUT])

                # s4 + sB: down horizontal (banded) + linear-path accumulate
                for mo, (h0, h1) in enumerate(((0, 128), (128, OUT))):
                    rows = h1 - h0
                    P4 = ps4.tile([128, 256], F32, name="p4")
                    for k in range(4):
                        o0, o1 = DWIN[k]
                        nc.tensor.matmul(
                            out=P4[:rows, o0:o1],
                            lhsT=Z[:, k, h0:h1],
                            rhs=D_sb(k), start=(k == 0), stop=False)
                    # linear path accumulates into the same PSUM group
                    nc.tensor.matmul(
                        out=P4[:rows, MARGIN:MARGIN + OUT],
                        lhsT=U1[:, h0:h1],
                        rhs=Mh_sb, start=False, stop=True)
                    O = op_.tile([128, OUT], F32)
                    nc.vector.tensor_copy(out=O[:rows, :],
                                          in_=P4[:rows, MARGIN:MARGIN + OUT])
                    nc.sync.dma_start(out=o_d[i, h0:h1, :], in_=O[:rows, :])

    nc.finalize()
    return nc


def _filter_matrices(up_filter, down_filter):
    fu = np.asarray(up_filter, dtype=np.float64)
    fd = np.asarray(down_filter, dtype=np.float64)
    i = np.arange(128)[:, None]
    o = np.arange(512)[None, :]
    t = 10 + o - 4 * i
    A = np.where((t >= 0) & (t < 24), fu[np.clip(t, 0, 23)], 0.0)
    s = np.arange(512)[:, None]
    o2 = np.arange(256)[None, :]
    t2 = 6 + 2 * o2 - s
    D = np.where((t2 >= 0) & (t2 < 12), fd[np.clip(t2, 0, 11)], 0.0)
    return A, D


def _pack_consts(up_filter, down_filter):
    A, D = _filter_matrices(up_filter, down_filter)
    cm = np.zeros((128, 2048), dtype=np.float16)
    cm[:, 0:512] = A.astype(np.float16)
    cm[:, 512:1024] = (A * (0.4 * SQRT2)).astype(np.float16)
    dw0 = 1024
    for k, (o0, o1) in enumerate(DWIN):
        cm[:, dw0 + 70 * k: dw0 + 70 * k + (o1 - o0)] = \
            D[128 * k:128 * (k + 1), o0:o1].astype(np.float16)
    Mv = A @ D
    mv0 = dw0 + 280
    cm[:, mv0: mv0 + 256] = Mv.astype(np.float16)
    mh0 = mv0 + 256
    cm[:, mh0: mh0 + 236] = (Mv * (0.6 * SQRT2))[:, 10:246].astype(np.float16)
    return cm


def _run(x, bias, up_filter, down_filter, trace=False):
    from concourse.bass_utils import run_bass_kernel_spmd

    if "nc" not in _cache:
        _cache["nc"] = _build_nc()
    nc = _cache["nc"]

    cm = _pack_consts(up_filter, down_filter)
    xf = np.ascontiguousarray(np.asarray(x, dtype=np.float32)
                              .reshape(NCORES * NIMG, H, W))
    bias = np.asarray(bias, dtype=np.float32)
    bias_full = np.tile(bias, (NCORES * NIMG) // bias.shape[0])

    in_maps = []
    for c in range(NCORES):
        in_maps.append({
            "x": xf[NIMG * c: NIMG * (c + 1)],
            "bias": np.ascontiguousarray(bias_full[NIMG * c: NIMG * (c + 1)]),
            "cm": cm,
        })
    res = run_bass_kernel_spmd(nc, in_maps, core_ids=list(range(NCORES)),
                               trace=trace)
    out = np.concatenate([res.results[c]["out"][None] for c in range(NCORES)], 0)
    out = out.reshape(4, 256, OUT, OUT)
    return out, res


def kernel(x, bias, up_filter, down_filter):
    out, _ = _run(x, bias, up_filter, down_filter, trace=False)
    return out


def kernel_traced(x, bias, up_filter, down_filter):
    return _run(x, bias, up_filter, down_filter, trace=True)
